# revision 27
# baseline (speedup 1.0000x reference)
"""Trainium2 Bass kernel for GatedActivation (gate-conv3d + sigmoid gating).

Reference computation (see problem):
  x: [2, 120, 48, 48, 48] f32   (channels = 32 scalar + 16*3 (l=1) + 8*5 (l=2))
  w_gate: [24, 120, 5, 5, 5] f32
  g = sigmoid(conv3d(x, w_gate, same padding))         # [2, 24, 48, 48, 48]
  out[:, 0:32]   = relu(x[:, 0:32])
  out[:, 32+3m+d]  = x[:, 32+3m+d]  * g[:, m]          (m in 0..15)
  out[:, 80+5m+d]  = x[:, 80+5m+d]  * g[:, 16+m]       (m in 0..7)

Sharding: 8 cores = batch(2) x X-split(4 slabs of 12 planes).  Each core gets a
host-prepared zero-padded fp16 input slab [120, 16, 52, 52] (= [C, Xin, Ypad,
Zpad]) and produces fp32 [120, 12, 48, 48].

On-chip algorithm per core (matmul operands in fp16; fp32 PSUM accumulate):
  - conv is computed as 25 accumulating matmuls per output tile, one per
    (kx, ky) tap, with the kz (z) taps folded into the stationary columns:
      stationary W'[(kx,ky)][cin, kz*24+o], moving x[cin, y-tile, zpad-window]
    -> PSUM[(kz*24+o), y, zpad].  Stationary is padded to 128 columns so the
    fast-weight-load path engages and LDWEIGHTS hides under streaming.
  - the z-tap sum, fused with the gate->channel broadcast: one PSUM->SBUF
    cast copy, then 5 accumulating matmuls with 0/1 selector weights that
    both sum the z-taps and replicate each gate onto its 3 or 5 non-scalar
    channels (the z-shift lives in each matmul's moving AP, which is
    uniform over partitions - the only legal way to do a shifted
    cross-partition-group reduction; replicate-then-sigmoid equals
    sigmoid-then-replicate elementwise).
  - sigmoid on ACT, gating multiply on VectorE over all 120 channels, relu
    overwrite of channels 0:32 on ACT, per-x-plane output DMA.
"""

import sys

if "/opt/trn_rl_repo" not in sys.path:
    sys.path.insert(0, "/opt/trn_rl_repo")

import numpy as np

B = 2
C = 120
S = 48          # spatial size
K = 5           # conv kernel size
PAD = 2
NXS = 4         # x-axis shards
XS = S // NXS   # 12 output x-planes per core
XIN = XS + 2 * PAD   # 16 input planes per core
SP = S + 2 * PAD     # 52, padded y/z extent
YT = 8          # y-tile rows per matmul
NYT = S // YT   # 6 y-tiles per plane
NCO = 24        # gate output channels
NSC = 32        # scalar (relu) channels
CP = 128        # stationary columns incl. pad (FWL wants 128)
CIN = 128       # contraction rows incl. pad (8 zero input channels)
N_CORES = 8

_CACHE = {}


def _build(reps=1, fixups=True, copies=True):
    import contextlib

    import concourse.tile as tile
    from concourse import bacc, mybir

    f32 = mybir.dt.float32
    f16 = mybir.dt.float16

    nc = bacc.Bacc("TRN2", target_bir_lowering=False, debug=False,
                   num_devices=N_CORES)
    xs_d = nc.dram_tensor("xs", [CIN, XIN, SP, SP], f16, kind="ExternalInput").ap()
    wst_d = nc.dram_tensor("wst", [CIN, K * K, CP], f16, kind="ExternalInput").ap()
    esum_d = nc.dram_tensor("esum", [CIN, K, CP], f16, kind="ExternalInput").ap()
    y_d = nc.dram_tensor("y", [C, XS, S, S], f32, kind="ExternalOutput").ap()

    with tile.TileContext(nc) as tc:
        with tc.tile_pool(name="wpool", bufs=1) as wpool, \
             tc.tile_pool(name="planes", bufs=8) as plane_pool, \
             tc.tile_pool(name="convps", bufs=3, space="PSUM") as conv_pool, \
             tc.tile_pool(name="sshift", bufs=3) as sshift_pool, \
             tc.tile_pool(name="gpreps", bufs=3, space="PSUM") as gpre_pool, \
             tc.tile_pool(name="gsig", bufs=3) as gsig_pool, \
             tc.tile_pool(name="outpl", bufs=2) as out_pool:

            wst_t = wpool.tile([CIN, K * K, CP], f16)
            nc.sync.dma_start(wst_t[:], wst_d[:])
            esum_t = wpool.tile([CIN, K, CP], f16)
            nc.sync.dma_start(esum_t[:], esum_d[:])

            planes = {}
            planes_f = {}

            def load_plane(s):
                t = plane_pool.tile([CIN, SP, SP], f16, tag="plane", name=f"plane{s}")
                nc.sync.dma_start(t[:], xs_d[:, s])
                planes[s] = t
                planes_f[s] = t[:].rearrange("p y z -> p (y z)")

            outplanes = {}

            def emit_fixup(p, k, cps):
                # move conv psum to sbuf (fp16 cast), then z-tap sum fused
                # with the gate->channel broadcast:
                # g_pre_rep[m, y, r] = sum_c cps[(c*24+gate(m)), y, r+c],
                # done as 5 accumulating matmuls with 0/1 selector weights
                # (the z-shift lives in each matmul's moving AP).  Replicate-
                # then-sigmoid equals sigmoid-then-replicate elementwise.
                ss = sshift_pool.tile([CIN, YT, SP], f16)
                nc.scalar.copy(ss[:].rearrange("p y z -> p (y z)"), cps[:, :])
                gpre = gpre_pool.tile([CP, YT, S], f32)
                for c in range(K):
                    nc.tensor.matmul(gpre[:], esum_t[:, c, :], ss[:, :, c:c + S],
                                     start=(c == 0), stop=(c == K - 1))
                gsig = gsig_pool.tile([C, YT, S], f16)
                nc.scalar.activation(gsig[:], gpre[0:C, :, :],
                                     mybir.ActivationFunctionType.Sigmoid)

                if k == 0:
                    outplanes[p] = out_pool.tile([C, S, S], f32, tag="outplane",
                                                 name=f"outplane{p}")
                op_t = outplanes[p]
                xc = planes[p + PAD]  # center plane (kx = PAD)
                ys = k * YT
                # full-range gating multiply (gate rows 0:32 are sigmoid(0)
                # garbage), then the relu overwrites channels 0:32.  Non-zero-
                # start partition accesses are limited to <=32 partitions, so
                # this avoids an 88-partition op at start 32.
                nc.vector.tensor_mul(
                    op_t[:, ys:ys + YT, :],
                    xc[0:C, ys + PAD:ys + PAD + YT, PAD:PAD + S],
                    gsig[:, :, :])
                nc.scalar.activation(
                    op_t[0:NSC, ys:ys + YT, :],
                    xc[0:NSC, ys + PAD:ys + PAD + YT, PAD:PAD + S],
                    mybir.ActivationFunctionType.Relu)
                if k == NYT - 1:
                    nc.sync.dma_start(y_d[:, p], op_t[:])
                    del outplanes[p]

            rep_ctx = tc.For_i(0, reps, 1) if reps > 1 else contextlib.nullcontext()
            with rep_ctx:
                for s in range(K):
                    load_plane(s)

                pending = []
                for p in range(XS):
                    for k in range(NYT):
                        if k == 0 and p + K < XIN:
                            load_plane(p + K)
                        cps = conv_pool.tile([CP, YT * SP], f32)
                        for a in range(K):
                            for b in range(K):
                                st = (k * YT + b) * SP
                                nc.tensor.matmul(
                                    cps[:],
                                    wst_t[:, a * K + b, :],
                                    planes_f[p + a][:, st:st + YT * SP],
                                    start=(a == 0 and b == 0),
                                    stop=(a == K - 1 and b == K - 1))
                        if fixups:
                            pending.append((p, k, cps))
                            if len(pending) > 2:
                                emit_fixup(*pending.pop(0))
                        elif copies:
                            ss = sshift_pool.tile([CIN, YT, SP], f16,
                                                  name=f"ssd{p}_{k}")
                            nc.vector.tensor_copy(
                                ss[:].rearrange("p y z -> p (y z)"), cps[:, :])
                if fixups:
                    for args in pending:
                        emit_fixup(*args)

    nc.compile()
    return nc


def _host_inputs(x, w_gate):
    """Build the 8 per-core input maps (matmul operands pre-cast to fp16)."""
    x = np.ascontiguousarray(x, dtype=np.float32)
    w_gate = np.ascontiguousarray(w_gate, dtype=np.float32)

    # stationary weights: Wst[i, a*K+b, c*24+o] = w_gate[o, i, a, b, c],
    # padded with 8 zero columns to 128 for the fast weight load path.
    wst = np.transpose(w_gate, (1, 2, 3, 4, 0)).reshape(C, K * K, K * NCO)
    wstp = np.zeros((CIN, K * K, CP), dtype=np.float16)
    wstp[:C, :, :K * NCO] = wst.astype(np.float16)

    # esum[i, c, m] = 1 iff i == c*24 + gate(m): z-tap selector fused with the
    # gate->channel broadcast (channels 0:32 have no gate -> zero columns).
    def gate_of(m):
        if 32 <= m < 80:
            return (m - 32) // 3
        if 80 <= m < 120:
            return 16 + (m - 80) // 5
        return None

    esum = np.zeros((CIN, K, CP), dtype=np.float16)
    for c in range(K):
        for m in range(C):
            o = gate_of(m)
            if o is not None:
                esum[c * NCO + o, c, m] = 1.0

    in_maps = []
    for i in range(N_CORES):
        b = i // NXS
        x0 = (i % NXS) * XS
        slab = np.zeros((CIN, XIN, SP, SP), dtype=np.float16)
        s0 = max(0, x0 - PAD)
        s1 = min(S, x0 + XS + PAD)
        d0 = s0 - (x0 - PAD)
        slab[:C, d0:d0 + (s1 - s0), PAD:PAD + S, PAD:PAD + S] = \
            x[b, :, s0:s1].astype(np.float16)
        in_maps.append({"xs": slab, "wst": wstp, "esum": esum})
    return in_maps


def kernel(x, w_gate):
    import time

    from concourse.bass_utils import run_bass_kernel_spmd

    if "nc" not in _CACHE:
        _CACHE["nc"] = _build()
    nc = _CACHE["nc"]

    in_maps = _host_inputs(x, w_gate)
    last_err = None
    for attempt in range(3):
        try:
            res = run_bass_kernel_spmd(nc, in_maps, core_ids=list(range(N_CORES)))
            break
        except Exception as e:  # transient NRT device wedges recover on retry
            last_err = e
            time.sleep(5.0)
    else:
        raise last_err
    kernel._last_results = res

    out = np.empty((B, C, S, S, S), dtype=np.float32)
    for i in range(N_CORES):
        b = i // NXS
        x0 = (i % NXS) * XS
        out[b, :, x0:x0 + XS] = res.results[i]["y"]
    return out


# revision 33
# speedup vs baseline: 1.1244x; 1.1244x over previous
"""Trainium2 Bass kernel for GatedActivation (gate-conv3d + sigmoid gating).

Reference computation (see problem):
  x: [2, 120, 48, 48, 48] f32   (channels = 32 scalar + 16*3 (l=1) + 8*5 (l=2))
  w_gate: [24, 120, 5, 5, 5] f32
  g = sigmoid(conv3d(x, w_gate, same padding))         # [2, 24, 48, 48, 48]
  out[:, 0:32]   = relu(x[:, 0:32])
  out[:, 32+3m+d]  = x[:, 32+3m+d]  * g[:, m]          (m in 0..15)
  out[:, 80+5m+d]  = x[:, 80+5m+d]  * g[:, 16+m]       (m in 0..7)

Sharding: 8 cores = batch(2) x X-split(4 slabs of 12 planes).  Each core gets a
host-prepared zero-padded fp16 input slab [120, 16, 52, 52] (= [C, Xin, Ypad,
Zpad]) and produces fp32 [120, 12, 48, 48].

On-chip algorithm per core (matmul operands in fp16; fp32 PSUM accumulate):
  - conv is computed as 25 accumulating matmuls per output tile, one per
    (kx, ky) tap, with the kz (z) taps folded into the stationary columns:
      stationary W'[(kx,ky)][cin, kz*24+o], moving x[cin, y-tile, zpad-window]
    -> PSUM[(kz*24+o), y, zpad].  Stationary is padded to 128 columns so the
    fast-weight-load path engages and LDWEIGHTS hides under streaming.
  - the z-tap sum, fused with the gate->channel broadcast: one PSUM->SBUF
    cast copy, then 5 accumulating matmuls with 0/1 selector weights that
    both sum the z-taps and replicate each gate onto its 3 or 5 non-scalar
    channels (the z-shift lives in each matmul's moving AP, which is
    uniform over partitions - the only legal way to do a shifted
    cross-partition-group reduction; replicate-then-sigmoid equals
    sigmoid-then-replicate elementwise).
  - sigmoid on ACT, gating multiply on VectorE over all 120 channels, relu
    overwrite of channels 0:32 on ACT, per-x-plane output DMA.
"""

import sys

if "/opt/trn_rl_repo" not in sys.path:
    sys.path.insert(0, "/opt/trn_rl_repo")

import numpy as np

B = 2
C = 120
S = 48          # spatial size
K = 5           # conv kernel size
PAD = 2
NXS = 4         # x-axis shards
XS = S // NXS   # 12 output x-planes per core
XIN = XS + 2 * PAD   # 16 input planes per core
SP = S + 2 * PAD     # 52, padded y/z extent
YT = 8          # y-tile rows per matmul
NYT = S // YT   # 6 y-tiles per plane
NCO = 24        # gate output channels
NSC = 32        # scalar (relu) channels
CP = 128        # stationary columns incl. pad (FWL wants 128)
CIN = 128       # contraction rows incl. pad (8 zero input channels)
N_CORES = 8

_CACHE = {}


def _build(reps=1, fixups=True, copies=True):
    import contextlib

    import concourse.tile as tile
    from concourse import bacc, mybir

    f32 = mybir.dt.float32
    f16 = mybir.dt.float16

    nc = bacc.Bacc("TRN2", target_bir_lowering=False, debug=False,
                   num_devices=N_CORES)
    xs_d = nc.dram_tensor("xs", [CIN, XIN, SP, SP], f16, kind="ExternalInput").ap()
    wst_d = nc.dram_tensor("wst", [CIN, K * K, CP], f16, kind="ExternalInput").ap()
    esum_d = nc.dram_tensor("esum", [CIN, CP], f16, kind="ExternalInput").ap()
    y_d = nc.dram_tensor("y", [C, XS, S, S], f32, kind="ExternalOutput").ap()

    with tile.TileContext(nc) as tc:
        with tc.tile_pool(name="wpool", bufs=1) as wpool, \
             tc.tile_pool(name="planes", bufs=8) as plane_pool, \
             tc.tile_pool(name="convps", bufs=3, space="PSUM") as conv_pool, \
             tc.tile_pool(name="sshift", bufs=3) as sshift_pool, \
             tc.tile_pool(name="sshift2", bufs=3) as sshift2_pool, \
             tc.tile_pool(name="gpreps", bufs=3, space="PSUM") as gpre_pool, \
             tc.tile_pool(name="gsig", bufs=3) as gsig_pool, \
             tc.tile_pool(name="outpl", bufs=2) as out_pool:

            wst_t = wpool.tile([CIN, K * K, CP], f16)
            nc.sync.dma_start(wst_t[:], wst_d[:])
            esum_t = wpool.tile([CIN, CP], f16)
            nc.sync.dma_start(esum_t[:], esum_d[:])

            planes = {}
            planes_f = {}

            def load_plane(s):
                t = plane_pool.tile([CIN, SP, SP], f16, tag="plane", name=f"plane{s}")
                nc.sync.dma_start(t[:], xs_d[:, s])
                planes[s] = t
                planes_f[s] = t[:].rearrange("p y z -> p (y z)")

            outplanes = {}

            def emit_fixup(p, k, cps):
                # move conv psum to sbuf (fp16 cast), then z-tap sum fused
                # with the gate->channel broadcast:
                # g_pre_rep[m, y, r] = sum_c cps[(c*24+gate(m)), y, r+c],
                # done as 5 accumulating matmuls with 0/1 selector weights
                # (the z-shift lives in each matmul's moving AP).  Replicate-
                # then-sigmoid equals sigmoid-then-replicate elementwise.
                ss = sshift_pool.tile([CIN, YT, SP], f16)
                nc.scalar.copy(ss[:].rearrange("p y z -> p (y z)"), cps[:, :])
                # realign the z-taps with 5 small SBUF->SBUF DMAs (DMA APs
                # have no 32-partition engine alignment restriction), so the
                # tap-sum + gate broadcast needs only ONE matmul.  Group c=4
                # includes the zero pad rows 120:128 (kept zero by the conv's
                # zero weight columns).
                ss2 = sshift2_pool.tile([CIN, YT, S], f16)
                for c in range(K):
                    np0 = 24 * c
                    np1 = 24 * (c + 1) if c < K - 1 else CIN
                    nc.sync.dma_start(ss2[np0:np1, :, :],
                                      ss[np0:np1, :, c:c + S])
                gpre = gpre_pool.tile([CP, YT, S], f32)
                nc.tensor.matmul(gpre[:], esum_t[:],
                                 ss2[:].rearrange("p y z -> p (y z)"),
                                 start=True, stop=True)
                gsig = gsig_pool.tile([C, YT, S], f16)
                nc.scalar.activation(gsig[:], gpre[0:C, :, :],
                                     mybir.ActivationFunctionType.Sigmoid)

                if k == 0:
                    outplanes[p] = out_pool.tile([C, S, S], f32, tag="outplane",
                                                 name=f"outplane{p}")
                op_t = outplanes[p]
                xc = planes[p + PAD]  # center plane (kx = PAD)
                ys = k * YT
                # full-range gating multiply (gate rows 0:32 are sigmoid(0)
                # garbage), then the relu overwrites channels 0:32.  Non-zero-
                # start partition accesses are limited to <=32 partitions, so
                # this avoids an 88-partition op at start 32.
                nc.vector.tensor_mul(
                    op_t[:, ys:ys + YT, :],
                    xc[0:C, ys + PAD:ys + PAD + YT, PAD:PAD + S],
                    gsig[:, :, :])
                nc.scalar.activation(
                    op_t[0:NSC, ys:ys + YT, :],
                    xc[0:NSC, ys + PAD:ys + PAD + YT, PAD:PAD + S],
                    mybir.ActivationFunctionType.Relu)
                if k == NYT - 1:
                    nc.sync.dma_start(y_d[:, p], op_t[:])
                    del outplanes[p]

            rep_ctx = tc.For_i(0, reps, 1) if reps > 1 else contextlib.nullcontext()
            with rep_ctx:
                for s in range(K):
                    load_plane(s)

                pending = []
                for p in range(XS):
                    for k in range(NYT):
                        if k == 0 and p + K < XIN:
                            load_plane(p + K)
                        cps = conv_pool.tile([CP, YT * SP], f32)
                        for a in range(K):
                            for b in range(K):
                                st = (k * YT + b) * SP
                                nc.tensor.matmul(
                                    cps[:],
                                    wst_t[:, a * K + b, :],
                                    planes_f[p + a][:, st:st + YT * SP],
                                    start=(a == 0 and b == 0),
                                    stop=(a == K - 1 and b == K - 1))
                        if fixups:
                            pending.append((p, k, cps))
                            if len(pending) > 2:
                                emit_fixup(*pending.pop(0))
                        elif copies:
                            ss = sshift_pool.tile([CIN, YT, SP], f16,
                                                  name=f"ssd{p}_{k}")
                            nc.vector.tensor_copy(
                                ss[:].rearrange("p y z -> p (y z)"), cps[:, :])
                if fixups:
                    for args in pending:
                        emit_fixup(*args)

    nc.compile()
    return nc


def _host_inputs(x, w_gate):
    """Build the 8 per-core input maps (matmul operands pre-cast to fp16)."""
    x = np.ascontiguousarray(x, dtype=np.float32)
    w_gate = np.ascontiguousarray(w_gate, dtype=np.float32)

    # stationary weights: Wst[i, a*K+b, c*24+o] = w_gate[o, i, a, b, c],
    # padded with 8 zero columns to 128 for the fast weight load path.
    wst = np.transpose(w_gate, (1, 2, 3, 4, 0)).reshape(C, K * K, K * NCO)
    wstp = np.zeros((CIN, K * K, CP), dtype=np.float16)
    wstp[:C, :, :K * NCO] = wst.astype(np.float16)

    # esum[i, c, m] = 1 iff i == c*24 + gate(m): z-tap selector fused with the
    # gate->channel broadcast (channels 0:32 have no gate -> zero columns).
    def gate_of(m):
        if 32 <= m < 80:
            return (m - 32) // 3
        if 80 <= m < 120:
            return 16 + (m - 80) // 5
        return None

    esum = np.zeros((CIN, CP), dtype=np.float16)
    for c in range(K):
        for m in range(C):
            o = gate_of(m)
            if o is not None:
                esum[c * NCO + o, m] = 1.0

    in_maps = []
    for i in range(N_CORES):
        b = i // NXS
        x0 = (i % NXS) * XS
        slab = np.zeros((CIN, XIN, SP, SP), dtype=np.float16)
        s0 = max(0, x0 - PAD)
        s1 = min(S, x0 + XS + PAD)
        d0 = s0 - (x0 - PAD)
        slab[:C, d0:d0 + (s1 - s0), PAD:PAD + S, PAD:PAD + S] = \
            x[b, :, s0:s1].astype(np.float16)
        in_maps.append({"xs": slab, "wst": wstp, "esum": esum})
    return in_maps


def kernel(x, w_gate):
    import time

    from concourse.bass_utils import run_bass_kernel_spmd

    if "nc" not in _CACHE:
        _CACHE["nc"] = _build()
    nc = _CACHE["nc"]

    in_maps = _host_inputs(x, w_gate)
    last_err = None
    for attempt in range(3):
        try:
            res = run_bass_kernel_spmd(nc, in_maps, core_ids=list(range(N_CORES)))
            break
        except Exception as e:  # transient NRT device wedges recover on retry
            last_err = e
            time.sleep(5.0)
    else:
        raise last_err
    kernel._last_results = res

    out = np.empty((B, C, S, S, S), dtype=np.float32)
    for i in range(N_CORES):
        b = i // NXS
        x0 = (i % NXS) * XS
        out[b, :, x0:x0 + XS] = res.results[i]["y"]
    return out
